# revision 1
# baseline (speedup 1.0000x reference)
"""nn_Block_67173288509603 on 8 TRN2 NeuronCores via Bass/Tile.

adaLN -> GQA block-causal attention (+RoPE) -> adaLN -> MoE (shared + top2-of-8).

Sharding (single SPMD program; all per-core differences flow through inputs
and collective replica-group semantics):
  core c in 0..7, b = c//4 (batch), g = c%4 (kv-head group), e = c (expert).
  - Phase A (token-parallel): each core adaLN-normalizes its quarter of
    tokens (512 rows), transposes, AllGather(groups of 4) -> h1^T[b] on
    every core of the b-group.  adaLN scale/shift GEMV is sharded 4-way per
    batch group and AllGathered.
  - Phase B (head-parallel): 4 q-heads + 1 kv head per core, full causal
    block attention in fp32, output projection partial, ReduceScatter
    (groups of 4) -> each core owns its token quarter of attn out.
  - Phase C: residual + adaLN2 on own quarter (fp32, exact), local router
    logits in fp32, tiny logit AllGather (8), h2^T AllGather (8, bf16).
  - Phase D: gates (exact fp32 top-2), shared expert (hidden-sliced 1/8),
    routed expert e=c dense-masked in bf16, fused into one PSUM
    accumulation, ReduceScatter(8) -> own token quarter, final residual.
Output: each core returns its [512, 1024] quarter; host concatenates.
"""

import os
import numpy as np
import ml_dtypes

import concourse.bass as bass
import concourse.mybir as mybir
import concourse.tile as tile
from concourse import bacc
from concourse.bass_utils import run_bass_kernel_spmd

F32 = mybir.dt.float32
BF16 = mybir.dt.bfloat16
AX = mybir.AxisListType
OP = mybir.AluOpType
ACT = mybir.ActivationFunctionType

B, T, C = 2, 2048, 1024
H, KVH, HD = 16, 4, 64
BLK = 128
THETA = 10000.0
E, TOPK = 8, 2
SH_H = 2048
EPS_LN = 1e-5
P = 128
NCORE = 8
TQ = 512              # tokens per core quarter
NT_Q = TQ // P        # 4 token tiles per quarter
CK = C // P           # 8 contraction tiles over C
NKB = T // BLK        # 16 kv blocks
N_TT = T // P         # 16 token tiles per batch

LAST_EXEC_NS = None

GROUPS_B = [[0, 1, 2, 3], [4, 5, 6, 7]]
GROUPS_ALL = [[0, 1, 2, 3, 4, 5, 6, 7]]


def _chunks_from(s, end, step=512):
    """512-aligned chunks covering [s, end); first chunk may be partial."""
    out = []
    while s < end:
        e = min(end, ((s // step) + 1) * step)
        out.append((s, e))
        s = e
    return out


def build_program():
    nc = bacc.Bacc("TRN2", target_bir_lowering=False, debug=False,
                   num_devices=NCORE)

    def din(name, shape, dt):
        return nc.dram_tensor(name, list(shape), dt, kind="ExternalInput").ap()

    t_x = din("x_q", [TQ, C], F32)
    t_temb = din("temb_b", [C, 1], F32)
    t_adaw = din("ada_w_s", [C, 1024], F32)
    t_adab = din("ada_b_s", [1, 1024], F32)
    t_wq = din("wq_s", [C, 256], F32)
    t_wkv = din("wkv_s", [C, 128], F32)
    t_wo = din("wo_s", [256, C], F32)
    t_cosq = din("cosq", [P, T], F32)
    t_sinq = din("sinq", [P, T], F32)
    t_idf = din("identf", [P, P], F32)
    t_swA = din("swA_s", [C, 512], BF16)
    t_sw2 = din("sw2_s", [256, C], BF16)
    t_rw1 = din("rw1_e", [C, 1024], BF16)
    t_rw2 = din("rw2_e", [C, 1024], BF16)
    t_rtw = din("router_w", [C, E], F32)
    t_rtb = din("router_bias", [1, E], F32)
    t_esel = din("esel", [1, E], F32)

    t_out = nc.dram_tensor("out", [TQ, C], F32, kind="ExternalOutput").ap()

    with tile.TileContext(nc) as tc:
        _build(tc, dict(
            x_q=t_x, temb_b=t_temb, ada_w_s=t_adaw, ada_b_s=t_adab,
            wq_s=t_wq, wkv_s=t_wkv, wo_s=t_wo, cosq=t_cosq, sinq=t_sinq,
            identf=t_idf, swA_s=t_swA, sw2_s=t_sw2, rw1_e=t_rw1,
            rw2_e=t_rw2, router_w=t_rtw, router_bias=t_rtb, esel=t_esel,
            out=t_out))
    nc.compile()
    return nc


def _build(tc, io):
    nc = tc.nc
    import os as _os
    from contextlib import ExitStack
    PHASES = _os.environ.get("KB_PHASES", "D")

    def _dummy_out(pool):
        for t in range(NT_Q):
            z = pool.tile([P, C], F32, name=f"dz_{t}", tag="dz", bufs=2)
            nc.vector.memset(z[:], 0.0)
            nc.sync.dma_start(io["out"][t * P:(t + 1) * P, :], z[:])

    top = ExitStack()
    with top:
        dram = top.enter_context(tc.tile_pool(name="dram", bufs=1, space="DRAM"))
        pers0 = top.enter_context(tc.tile_pool(name="pers0", bufs=1))

        # ---- collective buffers -------------------------------------------
        ag_ada_in = dram.tile([1, 1024], F32, name="ag_ada_in")
        ag_ada_out = dram.tile([4, 1024], F32, name="ag_ada_out")
        ag_h1_in = dram.tile([C, TQ], F32, name="ag_h1_in")
        ag_h1_out = dram.tile([4 * C, TQ], F32, name="ag_h1_out")
        rs_at_in = dram.tile([T, C], F32, name="rs_at_in")
        rs_at_out = dram.tile([TQ, C], F32, name="rs_at_out")
        ag_lg_in = dram.tile([E, TQ], F32, name="ag_lg_in")
        ag_lg_out = dram.tile([NCORE * E, TQ], F32, name="ag_lg_out",
                              addr_space="Shared")
        ag_h2_in = dram.tile([C, TQ], BF16, name="ag_h2_in")
        ag_h2_out = dram.tile([NCORE * C, TQ], BF16, name="ag_h2_out",
                              addr_space="Shared")
        rs_mo_in = dram.tile([B * T, C], BF16, name="rs_mo_in")
        rs_mo_out = dram.tile([TQ, C], BF16, name="rs_mo_out")
        scr_row = dram.tile([1, T], F32, name="scr_row")
        scr_g = dram.tile([1, B * T], BF16, name="scr_g")

        # ---- whole-kernel persistents -------------------------------------
        identf = pers0.tile([P, P], F32, name="identf", tag="identf")
        nc.sync.dma_start(identf[:], io["identf"][:])
        sc1 = pers0.tile([P, C], F32, name="sc1", tag="sc1")
        sh1 = pers0.tile([P, C], F32, name="sh1", tag="sh1")
        sc2 = pers0.tile([P, C], F32, name="sc2", tag="sc2")
        sh2 = pers0.tile([P, C], F32, name="sh2", tag="sh2")
        x2 = [pers0.tile([P, C], F32, name=f"x2_{t}", tag=f"x2_{t}")
              for t in range(NT_Q)]

        def recip_act(pool, out_ap, in_ap, name, power=-1.0):
            """out = in^power via Exp(power * Ln(in)); in must be > 0."""
            shp = list(in_ap.shape)
            t = pool.tile(shp, F32, name=f"{name}_lnr", tag="rc_ln", bufs=2)
            nc.scalar.activation(t[:], in_ap, ACT.Ln)
            nc.scalar.activation(out_ap, t[:], ACT.Exp, scale=power)

        def layernorm_tile(pool, x_sb, scv, shv, name):
            """x_sb [P, C] fp32 -> adaLN -> fp32 tile."""
            s1 = pool.tile([P, 1], F32, name=f"{name}_s1", tag="ln_s1", bufs=2)
            nc.vector.tensor_reduce(s1[:], x_sb[:], axis=AX.X, op=OP.add)
            mean = pool.tile([P, 1], F32, name=f"{name}_mean", tag="ln_mean",
                             bufs=2)
            nc.vector.tensor_scalar_mul(mean[:], s1[:], 1.0 / C)
            xc = pool.tile([P, C], F32, name=f"{name}_xc", tag="ln_xc", bufs=2)
            nc.vector.tensor_scalar_sub(xc[:], x_sb[:], mean[:])
            sq = pool.tile([P, C], F32, name=f"{name}_sq", tag="ln_sq", bufs=2)
            ssq = pool.tile([P, 1], F32, name=f"{name}_ssq", tag="ln_ssq",
                            bufs=2)
            nc.scalar.activation(sq[:], xc[:], ACT.Square, accum_out=ssq[:])
            varep = pool.tile([P, 1], F32, name=f"{name}_varep", tag="ln_ve",
                              bufs=2)
            nc.vector.tensor_scalar(varep[:], ssq[:], 1.0 / C, EPS_LN,
                                    op0=OP.mult, op1=OP.add)
            rstd = pool.tile([P, 1], F32, name=f"{name}_rstd", tag="ln_rstd",
                             bufs=2)
            recip_act(pool, rstd[:], varep[:], name, power=-0.5)
            hn = pool.tile([P, C], F32, name=f"{name}_hn", tag="ln_hn", bufs=2)
            nc.vector.tensor_scalar_mul(hn[:], xc[:], rstd[:])
            h = pool.tile([P, C], F32, name=f"{name}_h", tag="ln_h", bufs=2)
            nc.vector.tensor_mul(hn[:], hn[:], scv[:])
            nc.vector.tensor_add(h[:], hn[:], shv[:])
            return h

        # =====================================================================
        # Phase A: ada GEMV + LN1 on own quarter + transpose + AllGather
        # =====================================================================
        with tc.tile_pool(name="phA", bufs=1) as pa, \
             tc.tile_pool(name="phA_ps", bufs=2, space="PSUM") as pa_ps:
            temb_sb = []
            adaw_sb = []
            for k in range(CK):
                tt = pa.tile([P, 1], F32, name=f"temb_{k}", tag=f"temb{k}")
                nc.sync.dma_start(tt[:], io["temb_b"][k * P:(k + 1) * P, :])
                temb_sb.append(tt)
                wt = pa.tile([P, 1024], F32, name=f"adaw_{k}", tag=f"adaw{k}")
                nc.sync.dma_start(wt[:], io["ada_w_s"][k * P:(k + 1) * P, :])
                adaw_sb.append(wt)
            ada_sb = pa.tile([1, 1024], F32, name="ada_sb", tag="ada_sb")
            adab_sb = pa.tile([1, 1024], F32, name="adab_sb", tag="adab_sb")
            nc.sync.dma_start(adab_sb[:], io["ada_b_s"][:])
            for n in range(2):
                ps = pa_ps.tile([1, 512], F32, name="ada_ps", tag="ada_ps",
                                bufs=2)
                for k in range(CK):
                    nc.tensor.matmul(ps[:], temb_sb[k][:],
                                     adaw_sb[k][:, n * 512:(n + 1) * 512],
                                     start=(k == 0), stop=(k == CK - 1))
                nc.vector.tensor_add(ada_sb[:, n * 512:(n + 1) * 512], ps[:],
                                     adab_sb[:, n * 512:(n + 1) * 512])
            nc.sync.dma_start(ag_ada_in[:], ada_sb[:])
            nc.gpsimd.collective_compute(
                "AllGather", OP.bypass, replica_groups=GROUPS_B,
                ins=[ag_ada_in.opt()], outs=[ag_ada_out.opt()])
            if PHASES == "A0":
                _dummy_out(pa)
                return
            flat = ag_ada_out[:].rearrange("a b -> (a b)")
            for i, (dst, add1) in enumerate(((sc1, True), (sh1, False),
                                             (sc2, True), (sh2, False))):
                nc.sync.dma_start(
                    dst[:], flat[i * C:(i + 1) * C][None, :].to_broadcast([P, C]))
                if add1:
                    nc.vector.tensor_scalar_add(dst[:], dst[:], 1.0)
            if PHASES == "A1":
                _dummy_out(pa)
                return

        with tc.tile_pool(name="ln1", bufs=1) as p1, \
             tc.tile_pool(name="ln1_ps", bufs=2, space="PSUM") as p1_ps:
            for t in range(NT_Q):
                x_sb = p1.tile([P, C], F32, name=f"x_{t}", tag="x", bufs=2)
                nc.sync.dma_start(x_sb[:], io["x_q"][t * P:(t + 1) * P, :])
                h1 = layernorm_tile(p1, x_sb, sc1, sh1, f"l1_{t}")
                if PHASES == "A2":
                    continue
                for ci in range(CK):
                    tp = p1_ps.tile([P, P], F32, name="h1t_ps", tag="h1t_ps",
                                    bufs=2)
                    nc.tensor.transpose(tp[:], h1[:, ci * P:(ci + 1) * P],
                                        identf[:])
                    stg = p1.tile([P, P], F32, name=f"h1t_{t}_{ci}", tag="h1t",
                                  bufs=3)
                    nc.vector.tensor_copy(stg[:], tp[:])
                    nc.sync.dma_start(
                        ag_h1_in[ci * P:(ci + 1) * P, t * P:(t + 1) * P],
                        stg[:])
        if PHASES == "A2":
            with tc.tile_pool(name="dummy", bufs=1) as pdz:
                _dummy_out(pdz)
            return
        nc.gpsimd.collective_compute(
            "AllGather", OP.bypass, replica_groups=GROUPS_B,
            ins=[ag_h1_in.opt()], outs=[ag_h1_out.opt()])
        if PHASES == "A":
            with tc.tile_pool(name="dummy", bufs=1) as pdz:
                _dummy_out(pdz)
            return

        # =====================================================================
        # Phase B: attention for 4 heads / 1 kv head on own batch
        # =====================================================================
        pBstack = ExitStack()
        pB = pBstack.enter_context(tc.tile_pool(name="pB", bufs=1))
        qrope = [pB.tile([64, T], F32, name=f"qrope_{h}", tag=f"qrope{h}")
                 for h in range(4)]
        krope = pB.tile([64, T], F32, name="krope", tag="krope")
        vav = [pB.tile([P, 65], F32, name=f"vav_{kb}", tag=f"vav{kb}")
               for kb in range(NKB)]
        ytall = [pB.tile([P, T], F32, name=f"ytall_{i}", tag=f"ytall{i}")
                 for i in range(2)]

        with tc.tile_pool(name="qkv", bufs=1) as pq, \
             tc.tile_pool(name="qkv_ps", bufs=2, space="PSUM") as pq_ps:
            cosq = pq.tile([64, T], F32, name="cosq", tag="cosq")
            sinq = pq.tile([64, T], F32, name="sinq", tag="sinq")
            nc.sync.dma_start(cosq[:], io["cosq"][0:64, :])
            nc.sync.dma_start(sinq[:], io["sinq"][0:64, :])
            wq_sb = []
            wkv_sb = []
            for k in range(CK):
                wt = pq.tile([P, 256], F32, name=f"wq_{k}", tag=f"wq{k}")
                nc.sync.dma_start(wt[:], io["wq_s"][k * P:(k + 1) * P, :])
                wq_sb.append(wt)
                wt2 = pq.tile([P, P], F32, name=f"wkv_{k}", tag=f"wkv{k}")
                nc.sync.dma_start(wt2[:], io["wkv_s"][k * P:(k + 1) * P, :])
                wkv_sb.append(wt2)
            for n in range(T // 512):
                ns = n * 512
                h1c = []
                for k in range(CK):
                    hc = pq.tile([P, 512], F32, name=f"h1c_{n}_{k}",
                                 tag=f"h1c{k}", bufs=2)
                    nc.sync.dma_start(hc[:],
                                      ag_h1_out[n * C + k * P:
                                                n * C + (k + 1) * P, :])
                    h1c.append(hc)
                for h in range(4):
                    ps = pq_ps.tile([64, 512], F32, name="q_ps", tag="q_ps",
                                    bufs=2)
                    for k in range(CK):
                        nc.tensor.matmul(ps[:],
                                         wq_sb[k][:, h * 64:(h + 1) * 64],
                                         h1c[k][:],
                                         start=(k == 0), stop=(k == CK - 1))
                    qt = pq.tile([64, 512], F32, name="qt", tag="qt", bufs=3)
                    nc.vector.tensor_copy(qt[:], ps[:])
                    qs = pq.tile([64, 512], F32, name="qs", tag="qs", bufs=3)
                    for (do, so) in ((0, 32), (32, 0)):
                        nc.sync.dma_start(qs[do:do + 32, :], qt[so:so + 32, :])
                    nc.vector.tensor_mul(qt[:], qt[:],
                                         cosq[:, ns:ns + 512])
                    nc.vector.tensor_mul(qs[:], qs[:],
                                         sinq[:, ns:ns + 512])
                    nc.vector.tensor_add(qrope[h][:, ns:ns + 512],
                                         qt[:], qs[:])
                ps = pq_ps.tile([P, 512], F32, name="kv_ps", tag="kv_ps",
                                bufs=2)
                for k in range(CK):
                    nc.tensor.matmul(ps[:], wkv_sb[k][:], h1c[k][:],
                                     start=(k == 0), stop=(k == CK - 1))
                kvt = pq.tile([P, 512], F32, name="kvt", tag="kvt", bufs=2)
                nc.vector.tensor_copy(kvt[:], ps[:])
                ks = pq.tile([64, 512], F32, name="ks", tag="ks", bufs=2)
                for (do, so) in ((0, 32), (32, 0)):
                    nc.sync.dma_start(ks[do:do + 32, :], kvt[so:so + 32, :])
                kc = pq.tile([64, 512], F32, name="kc", tag="kc", bufs=2)
                nc.vector.tensor_mul(kc[:], kvt[0:64, :], cosq[:, ns:ns + 512])
                nc.vector.tensor_mul(ks[:], ks[:], sinq[:, ns:ns + 512])
                nc.vector.tensor_add(krope[:, ns:ns + 512], kc[:], ks[:])
                for j in range(4):
                    kb = n * 4 + j
                    vp = pq_ps.tile([P, 64], F32, name="v_ps", tag="v_ps",
                                    bufs=2)
                    nc.tensor.transpose(vp[:],
                                        kvt[64:P, j * P:(j + 1) * P],
                                        identf[64:P, 64:P])
                    nc.vector.memset(vav[kb][:, 64:65], 1.0)
                    nc.vector.tensor_copy(vav[kb][:, 0:64], vp[:])

        with tc.tile_pool(name="att", bufs=1) as pat, \
             tc.tile_pool(name="att_yt", bufs=1, space="PSUM") as pyt, \
             tc.tile_pool(name="att_st", bufs=3, space="PSUM") as pst:
            zl = pat.tile([1, 65], F32, name="zl", tag="zl")
            nc.vector.memset(zl[:], 0.0)
            zr = pat.tile([1, 512], F32, name="zr", tag="zr")
            nc.vector.memset(zr[:], 0.0)
            for h in range(4):
                i, ro = h // 2, (h % 2) * 64
                yt = pyt.tile([65, T], F32, name="yt_ps", tag="yt")
                for kb in range(NKB):
                    for (s, e) in _chunks_from(kb * P, T):
                        st = pst.tile([P, 512], F32, name="st_ps", tag="st",
                                      bufs=3)
                        nc.tensor.matmul(st[:, :e - s],
                                         krope[:, kb * P:(kb + 1) * P],
                                         qrope[h][:, s:e],
                                         start=True, stop=True)
                        pexp = pat.tile([P, 512], F32, name="pexp", tag="pexp",
                                        bufs=4)
                        nc.scalar.activation(pexp[:, :e - s], st[:, :e - s],
                                             ACT.Exp, scale=0.125)
                        nc.tensor.matmul(yt[:, s:e], vav[kb][:],
                                         pexp[:, :e - s],
                                         start=(kb == 0), stop=False)
                # close the per-region accumulation groups (adds zero)
                for j in range(4):
                    nc.tensor.matmul(yt[:, j * 512:(j + 1) * 512], zl[:],
                                     zr[:], start=False, stop=True)
                ytc = pat.tile([65, T], F32, name="ytc", tag="ytc", bufs=2)
                nc.vector.tensor_copy(ytc[:], yt[:])
                lrow = pat.tile([1, T], F32, name="lrow", tag="lrow", bufs=2)
                nc.sync.dma_start(lrow[:], ytc[64:65, :])
                rec = pat.tile([1, T], F32, name="rec", tag="rec", bufs=2)
                recip_act(pat, rec[:], lrow[:], f"at_{h}")
                nc.sync.dma_start(scr_row[:], rec[:])
                recb = pat.tile([64, T], F32, name="recb", tag="recb", bufs=2)
                nc.sync.dma_start(recb[:],
                                  scr_row[0, :][None, :].to_broadcast([64, T]))
                ytn = pat.tile([64, T], F32, name="ytn", tag="ytn", bufs=2)
                nc.vector.tensor_mul(ytn[:], ytc[0:64, :], recb[:])
                nc.sync.dma_start(ytall[i][ro:ro + 64, :], ytn[:])

        with tc.tile_pool(name="oproj", bufs=1) as po, \
             tc.tile_pool(name="oproj_ps", bufs=3, space="PSUM") as po_ps:
            wo_sb = []
            for k in range(2):
                wt = po.tile([P, C], F32, name=f"wo_{k}", tag=f"wo{k}")
                nc.sync.dma_start(wt[:], io["wo_s"][k * P:(k + 1) * P, :])
                wo_sb.append(wt)
            for t in range(N_TT):
                for n in range(2):
                    ps = po_ps.tile([P, 512], F32, name="o_ps", tag="o_ps",
                                    bufs=3)
                    for k in range(2):
                        nc.tensor.matmul(ps[:],
                                         ytall[k][:, t * P:(t + 1) * P],
                                         wo_sb[k][:, n * 512:(n + 1) * 512],
                                         start=(k == 0), stop=(k == 1))
                    o_sb = po.tile([P, 512], F32, name="o_sb", tag="o_sb",
                                   bufs=3)
                    nc.vector.tensor_copy(o_sb[:], ps[:])
                    nc.sync.dma_start(
                        rs_at_in[t * P:(t + 1) * P, n * 512:(n + 1) * 512],
                        o_sb[:])
        pBstack.close()
        nc.gpsimd.collective_compute(
            "ReduceScatter", OP.add, replica_groups=GROUPS_B,
            ins=[rs_at_in.opt()], outs=[rs_at_out.opt()])
        if PHASES == "B":
            with tc.tile_pool(name="dummy", bufs=1) as pdz:
                for t in range(NT_Q):
                    z = pdz.tile([P, C], F32, name=f"dz_{t}", tag="dz", bufs=2)
                    nc.sync.dma_start(z[:], rs_at_out[t * P:(t + 1) * P, :])
                    nc.sync.dma_start(io["out"][t * P:(t + 1) * P, :], z[:])
            return

        # =====================================================================
        # Phase C: residual + adaLN2 + local router logits (+ AGs)
        # =====================================================================
        with tc.tile_pool(name="ln2", bufs=1) as p2, \
             tc.tile_pool(name="ln2_ps", bufs=2, space="PSUM") as p2_ps:
            h2qT = [p2.tile([P, TQ], F32, name=f"h2qT_{k}", tag=f"h2qT{k}")
                    for k in range(CK)]
            for t in range(NT_Q):
                at_sb = p2.tile([P, C], F32, name=f"at_{t}", tag="at", bufs=2)
                nc.sync.dma_start(at_sb[:], rs_at_out[t * P:(t + 1) * P, :])
                xo_sb = p2.tile([P, C], F32, name=f"xo_{t}", tag="xo", bufs=2)
                nc.sync.dma_start(xo_sb[:], io["x_q"][t * P:(t + 1) * P, :])
                nc.vector.tensor_add(x2[t][:], xo_sb[:], at_sb[:])
                h2 = layernorm_tile(p2, x2[t], sc2, sh2, f"l2_{t}")
                for ci in range(CK):
                    tp = p2_ps.tile([P, P], F32, name="h2t_ps", tag="h2t_ps",
                                    bufs=2)
                    nc.tensor.transpose(tp[:], h2[:, ci * P:(ci + 1) * P],
                                        identf[:])
                    nc.vector.tensor_copy(h2qT[ci][:, t * P:(t + 1) * P],
                                          tp[:])
            for ci in range(CK):
                hb = p2.tile([P, TQ], BF16, name=f"h2b_{ci}", tag="h2b",
                             bufs=2)
                nc.vector.tensor_copy(hb[:], h2qT[ci][:])
                nc.sync.dma_start(ag_h2_in[ci * P:(ci + 1) * P, :], hb[:])
            rw_sb = []
            for k in range(CK):
                wt = p2.tile([P, E], F32, name=f"rw_{k}", tag=f"rw{k}")
                nc.sync.dma_start(wt[:], io["router_w"][k * P:(k + 1) * P, :])
                rw_sb.append(wt)
            lg_sb = p2.tile([E, TQ], F32, name="lg_sb", tag="lg_sb")
            lg_ps = p2_ps.tile([E, TQ], F32, name="lg_ps", tag="lg_ps")
            for k in range(CK):
                nc.tensor.matmul(lg_ps[:], rw_sb[k][:], h2qT[k][:],
                                 start=(k == 0), stop=(k == CK - 1))
            nc.vector.tensor_copy(lg_sb[:], lg_ps[:])
            nc.sync.dma_start(ag_lg_in[:], lg_sb[:])
        nc.gpsimd.collective_compute(
            "AllGather", OP.bypass, replica_groups=GROUPS_ALL,
            ins=[ag_lg_in.opt()], outs=[ag_lg_out.opt()])
        nc.gpsimd.collective_compute(
            "AllGather", OP.bypass, replica_groups=GROUPS_ALL,
            ins=[ag_h2_in.opt()], outs=[ag_h2_out.opt()])
        if PHASES == "C":
            with tc.tile_pool(name="dummy", bufs=1) as pdz:
                _dummy_out(pdz)
            return

        # =====================================================================
        # Phase D: gates + shared expert + routed expert e=c + RS
        # =====================================================================
        pD = top.enter_context(tc.tile_pool(name="pD", bufs=1))
        h2T = []
        for k in range(CK):
            tt = pD.tile([P, B * T], BF16, name=f"h2T_{k}", tag=f"h2T{k}")
            for r in range(NCORE):
                nc.sync.dma_start(
                    tt[:, r * TQ:(r + 1) * TQ],
                    ag_h2_out[r * C + k * P: r * C + (k + 1) * P, :])
            h2T.append(tt)
        gbc = pD.tile([P, B * T], BF16, name="gbc", tag="gbc")
        hT = [pD.tile([P, B * T], BF16, name=f"hT_{m}", tag=f"hT{m}")
              for m in range(2)]

        with tc.tile_pool(name="gates", bufs=1) as pg, \
             tc.tile_pool(name="gates_ps", bufs=2, space="PSUM") as pg_ps:
            bias_sb = pg.tile([P, E], F32, name="bias_sb", tag="bias_sb")
            nc.sync.dma_start(
                bias_sb[:], io["router_bias"][0, :][None, :].to_broadcast([P, E]))
            esel_sb = pg.tile([P, E], F32, name="esel_sb", tag="esel_sb")
            nc.sync.dma_start(
                esel_sb[:], io["esel"][0, :][None, :].to_broadcast([P, E]))
            gcols = pg.tile([P, 32], F32, name="gcols", tag="gcols")
            for r in range(NCORE):
                for tchk in range(NT_Q):
                    lg = pg.tile([E, P], F32, name="lg", tag="lg", bufs=2)
                    nc.sync.dma_start(
                        lg[:], ag_lg_out[r * E:(r + 1) * E,
                                         tchk * P:(tchk + 1) * P])
                    tp = pg_ps.tile([P, E], F32, name="lg_tp", tag="lg_tp",
                                    bufs=2)
                    nc.tensor.transpose(tp[:], lg[:], identf[0:E, 0:E])
                    s_sb = pg.tile([P, E], F32, name="s_sb", tag="s_sb",
                                   bufs=2)
                    nc.scalar.activation(s_sb[:], tp[:], ACT.Sigmoid)
                    sel = pg.tile([P, E], F32, name="sel", tag="sel", bufs=2)
                    nc.vector.tensor_add(sel[:], s_sb[:], bias_sb[:])
                    m8 = pg.tile([P, 8], F32, name="m8", tag="m8", bufs=2)
                    nc.vector.max(m8[:], sel[:])
                    mask = pg.tile([P, E], F32, name="mask", tag="mask",
                                   bufs=2)
                    nc.vector.tensor_scalar(mask[:], sel[:], m8[:, 1:2], None,
                                            op0=OP.is_ge)
                    sm = pg.tile([P, E], F32, name="sm", tag="sm", bufs=2)
                    nc.vector.tensor_mul(sm[:], s_sb[:], mask[:])
                    den = pg.tile([P, 1], F32, name="den", tag="den", bufs=2)
                    nc.vector.tensor_reduce(den[:], sm[:], axis=AX.X,
                                            op=OP.add)
                    nc.vector.tensor_scalar_add(den[:], den[:], 1e-9)
                    rden = pg.tile([P, 1], F32, name="rden", tag="rden",
                                   bufs=2)
                    recip_act(pg, rden[:], den[:], f"g_{r}_{tchk}")
                    ge = pg.tile([P, E], F32, name="ge", tag="ge", bufs=2)
                    nc.vector.tensor_scalar_mul(ge[:], sm[:], rden[:])
                    nc.vector.tensor_mul(ge[:], ge[:], esel_sb[:])
                    nc.vector.tensor_reduce(
                        gcols[:, r * 4 + tchk: r * 4 + tchk + 1], ge[:],
                        axis=AX.X, op=OP.add)
            gt_ps = pg_ps.tile([32, P], F32, name="gt_ps", tag="gt_ps")
            nc.tensor.transpose(gt_ps[:], gcols[:], identf[:])
            gt_sb = pg.tile([32, P], BF16, name="gt_sb", tag="gt_sb")
            nc.vector.tensor_copy(gt_sb[:], gt_ps[:])
            nc.sync.dma_start(
                scr_g[:].rearrange("a (b c) -> (a b) c", c=P), gt_sb[:])
            nc.sync.dma_start(gbc[:],
                              scr_g[0, :][None, :].to_broadcast([P, B * T]))

        # shared expert hT = silu(h2@sw1_s) * (h2@sw3_s), hidden slice 256
        with tc.tile_pool(name="shr", bufs=1) as psh, \
             tc.tile_pool(name="shr_ps", bufs=3, space="PSUM") as psh_ps:
            swA_sb = []
            for k in range(CK):
                wt = psh.tile([P, 512], BF16, name=f"swA_{k}", tag=f"swA{k}")
                nc.sync.dma_start(wt[:], io["swA_s"][k * P:(k + 1) * P, :])
                swA_sb.append(wt)
            for m in range(2):
                for n in range(B * T // 512):
                    a1 = psh_ps.tile([P, 512], F32, name="a1_ps", tag="a1_ps",
                                     bufs=2)
                    for k in range(CK):
                        nc.tensor.matmul(a1[:],
                                         swA_sb[k][:, m * P:(m + 1) * P],
                                         h2T[k][:, n * 512:(n + 1) * 512],
                                         start=(k == 0), stop=(k == CK - 1))
                    stmp = psh.tile([P, 512], BF16, name="stmp", tag="stmp",
                                    bufs=3)
                    nc.scalar.activation(stmp[:], a1[:], ACT.Silu)
                    a3 = psh_ps.tile([P, 512], F32, name="a3_ps", tag="a3_ps",
                                     bufs=2)
                    for k in range(CK):
                        nc.tensor.matmul(
                            a3[:], swA_sb[k][:, 256 + m * P:256 + (m + 1) * P],
                            h2T[k][:, n * 512:(n + 1) * 512],
                            start=(k == 0), stop=(k == CK - 1))
                    nc.vector.tensor_mul(hT[m][:, n * 512:(n + 1) * 512],
                                         stmp[:], a3[:])

        # routed expert (dense, gate-masked) fused with shared output matmul
        with tc.tile_pool(name="moe", bufs=1) as pm, \
             tc.tile_pool(name="moe_ps", bufs=3, space="PSUM") as pm_ps:
            rw1_sb = []
            rw2_sb = []
            for k in range(CK):
                w1 = pm.tile([P, 1024], BF16, name=f"rw1_{k}", tag=f"rw1{k}")
                nc.sync.dma_start(w1[:], io["rw1_e"][k * P:(k + 1) * P, :])
                rw1_sb.append(w1)
                w2 = pm.tile([P, 1024], BF16, name=f"rw2_{k}", tag=f"rw2{k}")
                nc.sync.dma_start(w2[:], io["rw2_e"][k * P:(k + 1) * P, :])
                rw2_sb.append(w2)
            sw2_sb = []
            for k in range(2):
                wt = pm.tile([P, C], BF16, name=f"sw2_{k}", tag=f"sw2{k}")
                nc.sync.dma_start(wt[:], io["sw2_s"][k * P:(k + 1) * P, :])
                sw2_sb.append(wt)
            NCHUNK = 8
            CHT = B * T // NCHUNK  # 512 tokens per chunk
            for tch in range(NCHUNK):
                hmid = [pm.tile([P, CHT], BF16, name=f"hm_{m}", tag=f"hm{m}",
                                bufs=2) for m in range(CK)]
                for m in range(CK):
                    ps = pm_ps.tile([P, 512], F32, name="w1_ps", tag="w1_ps",
                                    bufs=2)
                    col = tch * CHT
                    for k in range(CK):
                        nc.tensor.matmul(ps[:],
                                         rw1_sb[k][:, m * P:(m + 1) * P],
                                         h2T[k][:, col:col + CHT],
                                         start=(k == 0), stop=(k == CK - 1))
                    gl = pm.tile([P, 512], BF16, name="gl", tag="gl", bufs=3)
                    nc.scalar.activation(gl[:], ps[:], ACT.Gelu)
                    nc.vector.tensor_mul(hmid[m][:], gl[:],
                                         gbc[:, col:col + CHT])
                for tt in range(CHT // P):
                    gt = tch * (CHT // P) + tt  # global token tile
                    for n in range(2):
                        ps = pm_ps.tile([P, 512], F32, name="o2_ps",
                                        tag="o2_ps", bufs=2)
                        for k in range(2):
                            nc.tensor.matmul(
                                ps[:], hT[k][:, gt * P:(gt + 1) * P],
                                sw2_sb[k][:, n * 512:(n + 1) * 512],
                                start=(k == 0), stop=False)
                        for k in range(CK):
                            nc.tensor.matmul(
                                ps[:], hmid[k][:, tt * P:(tt + 1) * P],
                                rw2_sb[k][:, n * 512:(n + 1) * 512],
                                start=False, stop=(k == CK - 1))
                        mo = pm.tile([P, 512], BF16, name="mo", tag="mo",
                                     bufs=3)
                        nc.vector.tensor_copy(mo[:], ps[:])
                        nc.sync.dma_start(
                            rs_mo_in[gt * P:(gt + 1) * P,
                                     n * 512:(n + 1) * 512], mo[:])
        nc.gpsimd.collective_compute(
            "ReduceScatter", OP.add, replica_groups=GROUPS_ALL,
            ins=[rs_mo_in.opt()], outs=[rs_mo_out.opt()])

        # final residual
        with tc.tile_pool(name="fin", bufs=1) as pf:
            for t in range(NT_Q):
                mo_sb = pf.tile([P, C], BF16, name=f"mo_{t}", tag="fmo",
                                bufs=2)
                nc.sync.dma_start(mo_sb[:], rs_mo_out[t * P:(t + 1) * P, :])
                mo32 = pf.tile([P, C], F32, name=f"mo32_{t}", tag="fmo32",
                               bufs=2)
                nc.vector.tensor_copy(mo32[:], mo_sb[:])
                o_sb = pf.tile([P, C], F32, name=f"fo_{t}", tag="fo", bufs=2)
                nc.vector.tensor_add(o_sb[:], x2[t][:], mo32[:])
                nc.sync.dma_start(io["out"][t * P:(t + 1) * P, :], o_sb[:])


# =============================================================================
# host side
# =============================================================================

def _rope_tables():
    freqs = (1.0 / (THETA ** (np.arange(0, HD, 2, dtype=np.float64) / HD)))
    t = np.arange(T, dtype=np.float64)
    emb = np.outer(t, freqs)                       # [T, 32]
    cos = np.concatenate([np.cos(emb), np.cos(emb)], 1).T   # [64, T]
    sin = np.concatenate([np.sin(emb), np.sin(emb)], 1).T   # [64, T]
    sinS = sin.copy()
    sinS[0:32] = -sin[0:32]
    cosq = np.concatenate([cos, cos], 0).astype(np.float32)   # [128, T]
    sinq = np.concatenate([sinS, sinS], 0).astype(np.float32)
    return cosq, sinq


def _shard_inputs(inp):
    bf = ml_dtypes.bfloat16
    f32 = np.float32
    x = np.asarray(inp["x"], f32).reshape(B * T, C)
    t_emb = np.asarray(inp["t_emb"], f32)
    ada_cat = np.concatenate([np.asarray(inp["ada1_w"], f32),
                              np.asarray(inp["ada2_w"], f32)], 1)  # [C, 4096]
    adab_cat = np.concatenate([np.asarray(inp["ada1_b"], f32),
                               np.asarray(inp["ada2_b"], f32)])    # [4096]
    wq = np.asarray(inp["wq"], f32)
    wk = np.asarray(inp["wk"], f32)
    wv = np.asarray(inp["wv"], f32)
    wo = np.asarray(inp["wo"], f32)
    sw1 = np.asarray(inp["sw1"], f32)
    sw3 = np.asarray(inp["sw3"], f32)
    sw2 = np.asarray(inp["sw2"], f32)
    rw1 = np.asarray(inp["re_w1"], f32)
    rw2 = np.asarray(inp["re_w2"], f32)
    rtw = np.asarray(inp["router_w"], f32)
    rtb = np.asarray(inp["router_bias"], f32)
    cosq, sinq = _rope_tables()
    ident = np.eye(P, dtype=f32)

    in_maps = []
    for c in range(NCORE):
        b, g = c // 4, c % 4
        m = {
            "x_q": np.ascontiguousarray(x[c * TQ:(c + 1) * TQ]),
            "temb_b": np.ascontiguousarray(t_emb[b].reshape(C, 1)),
            "ada_w_s": np.ascontiguousarray(ada_cat[:, g * 1024:(g + 1) * 1024]),
            "ada_b_s": np.ascontiguousarray(
                adab_cat[g * 1024:(g + 1) * 1024].reshape(1, 1024)),
            "wq_s": np.ascontiguousarray(wq[:, 256 * g:256 * (g + 1)]),
            "wkv_s": np.ascontiguousarray(np.concatenate(
                [wk[:, 64 * g:64 * (g + 1)], wv[:, 64 * g:64 * (g + 1)]], 1)),
            "wo_s": np.ascontiguousarray(wo[256 * g:256 * (g + 1), :]),
            "cosq": cosq,
            "sinq": sinq,
            "identf": ident,
            "swA_s": np.ascontiguousarray(np.concatenate(
                [sw1[:, 256 * c:256 * (c + 1)],
                 sw3[:, 256 * c:256 * (c + 1)]], 1)).astype(bf),
            "sw2_s": np.ascontiguousarray(sw2[256 * c:256 * (c + 1), :]).astype(bf),
            "rw1_e": np.ascontiguousarray(rw1[c]).astype(bf),
            "rw2_e": np.ascontiguousarray(rw2[c]).astype(bf),
            "router_w": rtw,
            "router_bias": rtb.reshape(1, E),
            "esel": np.eye(E, dtype=f32)[c].reshape(1, E),
        }
        in_maps.append(m)
    return in_maps


_NC_CACHE = []


def _install_ntff_hook():
    """Provide antenv.axon_hooks (absent in this image) so trace=True works."""
    import sys
    import types
    try:
        import antenv
        if "antenv.axon_hooks" not in sys.modules:
            mod = types.ModuleType("antenv.axon_hooks")
            mod._hook = None

            def set_axon_ntff_profile_hook(h):
                mod._hook = h

            def get_axon_ntff_profile_hook():
                return mod._hook

            mod.set_axon_ntff_profile_hook = set_axon_ntff_profile_hook
            mod.get_axon_ntff_profile_hook = get_axon_ntff_profile_hook
            sys.modules["antenv.axon_hooks"] = mod
            antenv.axon_hooks = mod
        mod = sys.modules["antenv.axon_hooks"]
        if mod.get_axon_ntff_profile_hook() is None:
            from trn_agent_boot.trn_boot import _ntff_profile_via_ctypes
            hook = _ntff_profile_via_ctypes("/opt/axon/libaxon_pjrt.so")
            if hook is not None:
                mod.set_axon_ntff_profile_hook(hook)
        import concourse.bass_utils as bu
        bu.upload_artifacts = lambda d: d
        return True
    except Exception:
        return False


def kernel(**inputs):
    global LAST_EXEC_NS
    if not _NC_CACHE:
        _NC_CACHE.append(build_program())
    nc = _NC_CACHE[0]
    in_maps = _shard_inputs(inputs)
    trace = bool(int(os.environ.get("KB_TRACE", "0")))
    if trace:
        trace = _install_ntff_hook()
    res = None
    if trace:
        try:
            res = run_bass_kernel_spmd(nc, in_maps,
                                       core_ids=list(range(NCORE)),
                                       trace=True,
                                       tmpdir=os.environ.get("KB_TRACE_DIR"))
        except Exception as e:
            print(f"traced run failed ({e!r}); falling back to untraced")
            res = None
    if res is None:
        res = run_bass_kernel_spmd(nc, in_maps, core_ids=list(range(NCORE)))
    LAST_EXEC_NS = res.exec_time_ns
    out = np.concatenate([res.results[c]["out"].astype(np.float32)
                          for c in range(NCORE)], 0)
    return out.reshape(B, T, C)



# revision 11
# speedup vs baseline: 1.5537x; 1.5537x over previous
"""nn_Block_67173288509603 on 8 TRN2 NeuronCores via Bass/Tile.

adaLN -> GQA block-causal attention (+RoPE) -> adaLN -> MoE (shared + top2-of-8).

v2: bf16 matmul paths + slab-pipelined chunked collectives.

Sharding (single SPMD program; per-core differences flow through inputs and
replica-group semantics):
  core c in 0..7, b = c//4 (batch), g = c%4 (kv-head group), e = c (expert).
  Token ownership is INTERLEAVED: core c owns token tiles {4j+g : j=0..3} of
  batch b (tile = 128 tokens).  Slab j = global tokens [j*512,(j+1)*512) of a
  batch = the j-th owned tile of each core in the batch group.

  - Phase A: ada GEMV (bf16, 4-way sharded + tiny AllGather), LN1 stats early,
    per-owned-tile transpose -> 4 chunked AllGathers (bf16) of h1^T.
  - Phase B (per slab j): QKV projection (bf16) + RoPE (rot-half via +-1
    permutation matmul), block-causal attention for 4 q-heads/1 kv-head in
    bf16 (exp trick, fused denominator row), output projection partial,
    chunked ReduceScatter (bf16, groups of 4) -> own tile j of attn out.
  - Phase C (per owned tile j): residual + adaLN2 (fp32), transpose, chunked
    AllGather of h2^T (bf16, all 8); fp32 router logits + exact top-2 gates on
    owner; tiny chunked AllGather of per-expert gate rows.
  - Phase D (per chunk j = 1024 tokens): shared expert (hidden 1/8) + routed
    expert e=c dense-masked, fused PSUM accumulation, chunked ReduceScatter
    (all 8) -> own tile, final residual.
Output: each core returns its [512, 1024] interleaved quarter; host scatters.
"""

import os
import numpy as np
import ml_dtypes

import concourse.bass as bass
import concourse.mybir as mybir
import concourse.tile as tile
from concourse import bacc
from concourse.bass_utils import run_bass_kernel_spmd

F32 = mybir.dt.float32
BF16 = mybir.dt.bfloat16
AX = mybir.AxisListType
OP = mybir.AluOpType
ACT = mybir.ActivationFunctionType

B, T, C = 2, 2048, 1024
H, KVH, HD = 16, 4, 64
BLK = 128
THETA = 10000.0
E, TOPK = 8, 2
SH_H = 2048
EPS_LN = 1e-5
P = 128
NCORE = 8
TQ = 512              # tokens per core quarter
NT_Q = TQ // P        # 4 owned tiles / slabs
CK = C // P           # 8 contraction tiles over C
NKB = T // BLK        # 16 kv blocks

LAST_EXEC_NS = None

GROUPS_B = [[0, 1, 2, 3], [4, 5, 6, 7]]
GROUPS_ALL = [[0, 1, 2, 3, 4, 5, 6, 7]]


def build_program():
    nc = bacc.Bacc("TRN2", target_bir_lowering=False, debug=False,
                   num_devices=NCORE)

    def din(name, shape, dt):
        return nc.dram_tensor(name, list(shape), dt, kind="ExternalInput").ap()

    io = dict(
        x_q=din("x_q", [TQ, C], F32),
        temb_b=din("temb_b", [C, 1], F32),
        ada_w_s=din("ada_w_s", [C, 1024], BF16),
        ada_b_s=din("ada_b_s", [1, 1024], F32),
        wq_s=din("wq_s", [C, 256], BF16),
        wkv_s=din("wkv_s", [C, 128], BF16),
        wo_s=din("wo_s", [256, C], BF16),
        cosq=din("cosq", [64, T], F32),
        sinq=din("sinq", [64, T], F32),
        identf=din("identf", [P, P], F32),
        rotp=din("rotp", [64, 64], BF16),
        swA_s=din("swA_s", [C, 512], BF16),
        sw2_s=din("sw2_s", [256, C], BF16),
        rw1_e=din("rw1_e", [C, 1024], BF16),
        rw2_e=din("rw2_e", [C, 1024], BF16),
        router_w=din("router_w", [C, E], F32),
        router_bias=din("router_bias", [1, E], F32),
        out=nc.dram_tensor("out", [TQ, C], F32, kind="ExternalOutput").ap(),
    )

    with tile.TileContext(nc) as tc:
        _build(tc, io)
    nc.compile()
    return nc


def _build(tc, io):
    nc = tc.nc
    from contextlib import ExitStack

    top = ExitStack()
    with top:
        dram = top.enter_context(tc.tile_pool(name="dram", bufs=1, space="DRAM"))
        pers = top.enter_context(tc.tile_pool(name="pers", bufs=1))
        pw = top.enter_context(tc.tile_pool(name="pw", bufs=1))

        # ---- chunked collective buffers -----------------------------------
        ag_ada_in = dram.tile([1, 1024], F32, name="ag_ada_in")
        ag_ada_out = dram.tile([4, 1024], F32, name="ag_ada_out")
        h1ag_in = [dram.tile([C, P], BF16, name=f"h1ag_in_{j}")
                   for j in range(NT_Q)]
        h1ag_out = [dram.tile([4 * C, P], BF16, name=f"h1ag_out_{j}")
                    for j in range(NT_Q)]
        rsat_in = [dram.tile([TQ, C], BF16, name=f"rsat_in_{j}")
                   for j in range(NT_Q)]
        rsat_out = [dram.tile([P, C], BF16, name=f"rsat_out_{j}")
                    for j in range(NT_Q)]
        h2ag_in = [dram.tile([C, P], BF16, name=f"h2ag_in_{j}")
                   for j in range(NT_Q)]
        h2ag_out = [dram.tile([NCORE * C, P], BF16, name=f"h2ag_out_{j}",
                              addr_space="Shared") for j in range(NT_Q)]
        gag_in = [dram.tile([E, P], BF16, name=f"gag_in_{j}")
                  for j in range(NT_Q)]
        gag_out = [dram.tile([NCORE, P], BF16, name=f"gag_out_{j}")
                   for j in range(NT_Q)]
        rsmo_in = [dram.tile([NCORE * P, C], BF16, name=f"rsmo_in_{j}")
                   for j in range(NT_Q)]
        rsmo_out = [dram.tile([P, C], BF16, name=f"rsmo_out_{j}")
                    for j in range(NT_Q)]

        # ---- whole-kernel persistents -------------------------------------
        identf = pers.tile([P, P], F32, name="identf", tag="identf")
        nc.sync.dma_start(identf[:], io["identf"][:])
        rotp = pers.tile([64, 64], BF16, name="rotp", tag="rotp")
        nc.sync.dma_start(rotp[:], io["rotp"][:])
        sc1 = pers.tile([P, C], F32, name="sc1", tag="sc1")
        sh1 = pers.tile([P, C], F32, name="sh1", tag="sh1")
        sc2 = pers.tile([P, C], F32, name="sc2", tag="sc2")
        sh2 = pers.tile([P, C], F32, name="sh2", tag="sh2")
        x2 = [pers.tile([P, C], F32, name=f"x2_{t}", tag=f"x2_{t}")
              for t in range(NT_Q)]
        ones1 = pers.tile([1, 64], BF16, name="ones1", tag="ones1")
        nc.vector.memset(ones1[:], 1.0)
        zl = pers.tile([1, 65], BF16, name="zl", tag="zl")
        nc.vector.memset(zl[:], 0.0)
        zr = pers.tile([1, 512], BF16, name="zr", tag="zr")
        nc.vector.memset(zr[:], 0.0)
        bias_bc = pers.tile([P, E], F32, name="bias_bc", tag="bias_bc")
        nc.sync.dma_start(
            bias_bc[:], io["router_bias"][0, :][None, :].to_broadcast([P, E]))
        rtw = [pers.tile([P, E], F32, name=f"rtw_{k}", tag=f"rtw{k}")
               for k in range(CK)]
        for k in range(CK):
            nc.sync.dma_start(rtw[k][:], io["router_w"][k * P:(k + 1) * P, :])

        # ---- weight prefetch (phase B + D), DMAs issued up front ----------
        cosq = pw.tile([64, T], F32, name="cosq", tag="cosq")
        sinq = pw.tile([64, T], F32, name="sinq", tag="sinq")
        nc.sync.dma_start(cosq[:], io["cosq"][:])
        nc.sync.dma_start(sinq[:], io["sinq"][:])
        wq_sb, wkv_sb, swA_sb, rw1_sb, rw2_sb = [], [], [], [], []
        for k in range(CK):
            w = pw.tile([P, 256], BF16, name=f"wq_{k}", tag=f"wq{k}")
            nc.sync.dma_start(w[:], io["wq_s"][k * P:(k + 1) * P, :])
            wq_sb.append(w)
            w = pw.tile([P, P], BF16, name=f"wkv_{k}", tag=f"wkv{k}")
            nc.sync.dma_start(w[:], io["wkv_s"][k * P:(k + 1) * P, :])
            wkv_sb.append(w)
            w = pw.tile([P, 512], BF16, name=f"swA_{k}", tag=f"swA{k}")
            nc.sync.dma_start(w[:], io["swA_s"][k * P:(k + 1) * P, :])
            swA_sb.append(w)
            w = pw.tile([P, 1024], BF16, name=f"rw1_{k}", tag=f"rw1{k}")
            nc.sync.dma_start(w[:], io["rw1_e"][k * P:(k + 1) * P, :])
            rw1_sb.append(w)
            w = pw.tile([P, 1024], BF16, name=f"rw2_{k}", tag=f"rw2{k}")
            nc.sync.dma_start(w[:], io["rw2_e"][k * P:(k + 1) * P, :])
            rw2_sb.append(w)
        wo_sb, sw2_sb = [], []
        for k in range(2):
            w = pw.tile([P, C], BF16, name=f"wo_{k}", tag=f"wo{k}")
            nc.sync.dma_start(w[:], io["wo_s"][k * P:(k + 1) * P, :])
            wo_sb.append(w)
            w = pw.tile([P, C], BF16, name=f"sw2_{k}", tag=f"sw2{k}")
            nc.sync.dma_start(w[:], io["sw2_s"][k * P:(k + 1) * P, :])
            sw2_sb.append(w)

        def recip(pool, out_ap, in_ap, name, power=-1.0):
            """out = in^power via Exp(power * Ln(in)); in must be > 0."""
            shp = list(in_ap.shape)
            t = pool.tile(shp, F32, name=f"{name}_lnr", tag="rc_ln", bufs=1)
            nc.scalar.activation(t[:], in_ap, ACT.Ln)
            nc.scalar.activation(out_ap, t[:], ACT.Exp, scale=power)

        # =====================================================================
        # Phase A
        # =====================================================================
        pa_stack = ExitStack()
        pa = pa_stack.enter_context(tc.tile_pool(name="pa", bufs=1))
        pa_ps = pa_stack.enter_context(tc.tile_pool(name="pa_ps", bufs=2,
                                                    space="PSUM"))
        # x loads + LN1 stats (independent of ada result)
        xc = [pa.tile([P, C], F32, name=f"xc_{t}", tag=f"xc{t}")
              for t in range(NT_Q)]
        rstd1 = [pa.tile([P, 1], F32, name=f"rstd1_{t}", tag=f"rstd1{t}")
                 for t in range(NT_Q)]

        def ln_stats(pool, x_sb, xc_t, rstd_t, name):
            s1 = pool.tile([P, 1], F32, name=f"{name}_s1", tag="ln_s1", bufs=2)
            nc.vector.tensor_reduce(s1[:], x_sb[:], axis=AX.X, op=OP.add)
            mean = pool.tile([P, 1], F32, name=f"{name}_mean", tag="ln_mean",
                             bufs=2)
            nc.vector.tensor_scalar_mul(mean[:], s1[:], 1.0 / C)
            nc.vector.tensor_scalar_sub(xc_t[:], x_sb[:], mean[:])
            sq = pool.tile([P, C], F32, name=f"{name}_sq", tag="ln_sq", bufs=1)
            ssq = pool.tile([P, 1], F32, name=f"{name}_ssq", tag="ln_ssq",
                            bufs=2)
            nc.scalar.activation(sq[:], xc_t[:], ACT.Square, accum_out=ssq[:])
            varep = pool.tile([P, 1], F32, name=f"{name}_ve", tag="ln_ve",
                              bufs=2)
            nc.vector.tensor_scalar(varep[:], ssq[:], 1.0 / C, EPS_LN,
                                    op0=OP.mult, op1=OP.add)
            recip(pool, rstd_t[:], varep[:], name, power=-0.5)

        for t in range(NT_Q):
            nc.sync.dma_start(x2[t][:], io["x_q"][t * P:(t + 1) * P, :])
            ln_stats(pa, x2[t], xc[t], rstd1[t], f"l1_{t}")

        # ada GEMV (bf16, own 1024-slice) + AllGather over batch group
        temb_bf = []
        adaw_sb = []
        for k in range(CK):
            tt = pa.tile([P, 1], F32, name=f"temb_{k}", tag=f"temb{k}")
            nc.sync.dma_start(tt[:], io["temb_b"][k * P:(k + 1) * P, :])
            tb = pa.tile([P, 1], BF16, name=f"tembb_{k}", tag=f"tembb{k}")
            nc.vector.tensor_copy(tb[:], tt[:])
            temb_bf.append(tb)
            wt = pa.tile([P, 1024], BF16, name=f"adaw_{k}", tag=f"adaw{k}")
            nc.sync.dma_start(wt[:], io["ada_w_s"][k * P:(k + 1) * P, :])
            adaw_sb.append(wt)
        adab_sb = pa.tile([1, 1024], F32, name="adab_sb", tag="adab_sb")
        nc.sync.dma_start(adab_sb[:], io["ada_b_s"][:])
        ada_sb = pa.tile([1, 1024], F32, name="ada_sb", tag="ada_sb")
        for n in range(2):
            ps = pa_ps.tile([1, 512], F32, name="ada_ps", tag="ada_ps", bufs=2)
            for k in range(CK):
                nc.tensor.matmul(ps[:], temb_bf[k][:],
                                 adaw_sb[k][:, n * 512:(n + 1) * 512],
                                 start=(k == 0), stop=(k == CK - 1))
            nc.vector.tensor_add(ada_sb[:, n * 512:(n + 1) * 512], ps[:],
                                 adab_sb[:, n * 512:(n + 1) * 512])
        nc.sync.dma_start(ag_ada_in[:], ada_sb[:])
        nc.gpsimd.collective_compute(
            "AllGather", OP.bypass, replica_groups=GROUPS_B,
            ins=[ag_ada_in.opt()], outs=[ag_ada_out.opt()])
        flat = ag_ada_out[:].rearrange("a b -> (a b)")
        for i, (dst, add1) in enumerate(((sc1, True), (sh1, False),
                                         (sc2, True), (sh2, False))):
            nc.sync.dma_start(
                dst[:], flat[i * C:(i + 1) * C][None, :].to_broadcast([P, C]))
            if add1:
                nc.vector.tensor_scalar_add(dst[:], dst[:], 1.0)

        # LN1 apply + transpose + chunked h1 AllGather
        for t in range(NT_Q):
            hn = pa.tile([P, C], F32, name=f"hn_{t}", tag="hn", bufs=2)
            nc.vector.tensor_scalar_mul(hn[:], xc[t][:], rstd1[t][:])
            nc.vector.tensor_mul(hn[:], hn[:], sc1[:])
            h1 = pa.tile([P, C], F32, name=f"h1_{t}", tag="h1", bufs=2)
            nc.vector.tensor_add(h1[:], hn[:], sh1[:])
            for k in range(CK):
                tp = pa_ps.tile([P, P], F32, name="h1t_ps", tag="h1t_ps",
                                bufs=2)
                nc.tensor.transpose(tp[:], h1[:, k * P:(k + 1) * P], identf[:])
                stg = pa.tile([P, P], BF16, name=f"h1t_{t}_{k}", tag="h1t",
                              bufs=3)
                nc.scalar.copy(stg[:], tp[:])
                nc.sync.dma_start(h1ag_in[t][k * P:(k + 1) * P, :], stg[:])
            nc.gpsimd.collective_compute(
                "AllGather", OP.bypass, replica_groups=GROUPS_B,
                ins=[h1ag_in[t].opt()], outs=[h1ag_out[t].opt()])
        pa_stack.close()

        # =====================================================================
        # Phase B / C / D interleaved emission
        # =====================================================================
        pc = top.enter_context(tc.tile_pool(name="pc", bufs=1))
        pc_ps = top.enter_context(tc.tile_pool(name="pc_ps", bufs=2,
                                               space="PSUM"))

        def phase_c(j):
            """residual + LN2 + transpose/AG + router logits/gates + AG."""
            at_bf = pc.tile([P, C], BF16, name=f"at_{j}", tag="at", bufs=2)
            nc.sync.dma_start(at_bf[:], rsat_out[j][:])
            at32 = pc.tile([P, C], F32, name=f"at32_{j}", tag="at32", bufs=1)
            nc.vector.tensor_copy(at32[:], at_bf[:])
            nc.vector.tensor_add(x2[j][:], x2[j][:], at32[:])
            xc2 = pc.tile([P, C], F32, name=f"xc2_{j}", tag="xc2", bufs=1)
            rstd = pc.tile([P, 1], F32, name=f"rstd2_{j}", tag="rstd2",
                           bufs=2)
            ln_stats(pc, x2[j], xc2, rstd, f"l2_{j}")
            hn = pc.tile([P, C], F32, name=f"hn2_{j}", tag="hn2", bufs=1)
            nc.vector.tensor_scalar_mul(hn[:], xc2[:], rstd[:])
            nc.vector.tensor_mul(hn[:], hn[:], sc2[:])
            h2 = pc.tile([P, C], F32, name=f"h2_{j}", tag="h2", bufs=1)
            nc.vector.tensor_add(h2[:], hn[:], sh2[:])
            # transpose h2 tiles; bf16 copies feed the AG, fp32 copies feed
            # per-k router-logit partial matmuls accumulated in SBUF.
            lg_sb = pc.tile([E, P], F32, name=f"lg_{j}", tag="lg", bufs=2)
            for k in range(CK):
                tp = pc_ps.tile([P, P], F32, name="h2t_ps", tag="c_ps",
                                bufs=2)
                nc.tensor.transpose(tp[:], h2[:, k * P:(k + 1) * P],
                                    identf[:])
                s32 = pc.tile([P, P], F32, name=f"h2t32_{j}_{k}", tag="h2t32",
                              bufs=2)
                nc.vector.tensor_copy(s32[:], tp[:])
                sbf = pc.tile([P, P], BF16, name=f"h2tb_{j}_{k}", tag="h2tb",
                              bufs=3)
                nc.scalar.copy(sbf[:], tp[:])
                nc.sync.dma_start(h2ag_in[j][k * P:(k + 1) * P, :], sbf[:])
                lgp = pc_ps.tile([P, P], F32, name="lgp_ps", tag="c_ps",
                                 bufs=2)
                nc.tensor.matmul(lgp[0:E, :], rtw[k][:], s32[:],
                                 start=True, stop=True)
                lgs = pc.tile([E, P], F32, name="lgs", tag="lgs", bufs=2)
                nc.scalar.copy(lgs[:], lgp[0:E, :])
                if k == 0:
                    nc.vector.tensor_copy(lg_sb[:], lgs[:])
                else:
                    nc.vector.tensor_add(lg_sb[:], lg_sb[:], lgs[:])
            nc.gpsimd.collective_compute(
                "AllGather", OP.bypass, replica_groups=GROUPS_ALL,
                ins=[h2ag_in[j].opt()], outs=[h2ag_out[j].opt()])
            tp2 = pc_ps.tile([P, P], F32, name="lgT_ps", tag="c_ps", bufs=2)
            nc.tensor.transpose(tp2[:, 0:E], lg_sb[:], identf[0:E, 0:E])
            s_sb = pc.tile([P, E], F32, name=f"s_{j}", tag="s_sb", bufs=2)
            nc.scalar.activation(s_sb[:], tp2[:, 0:E], ACT.Sigmoid)
            sel = pc.tile([P, E], F32, name=f"sel_{j}", tag="sel", bufs=2)
            nc.vector.tensor_add(sel[:], s_sb[:], bias_bc[:])
            m8 = pc.tile([P, 8], F32, name=f"m8_{j}", tag="m8", bufs=2)
            nc.vector.max(m8[:], sel[:])
            mask = pc.tile([P, E], F32, name=f"mask_{j}", tag="mask", bufs=2)
            nc.vector.tensor_scalar(mask[:], sel[:], m8[:, 1:2], None,
                                    op0=OP.is_ge)
            sm = pc.tile([P, E], F32, name=f"sm_{j}", tag="sm", bufs=2)
            nc.vector.tensor_mul(sm[:], s_sb[:], mask[:])
            den = pc.tile([P, 1], F32, name=f"den_{j}", tag="den", bufs=2)
            nc.vector.tensor_reduce(den[:], sm[:], axis=AX.X, op=OP.add)
            nc.vector.tensor_scalar_add(den[:], den[:], 1e-9)
            rden = pc.tile([P, 1], F32, name=f"rden_{j}", tag="rden", bufs=2)
            recip(pc, rden[:], den[:], f"g_{j}")
            ge = pc.tile([P, E], F32, name=f"ge_{j}", tag="ge", bufs=2)
            nc.vector.tensor_scalar_mul(ge[:], sm[:], rden[:])
            gT_ps = pc_ps.tile([P, P], F32, name="gT_ps", tag="c_ps", bufs=2)
            nc.tensor.transpose(gT_ps[0:E, :], ge[:], identf[:])
            gT_bf = pc.tile([E, P], BF16, name=f"gT_{j}", tag="gT", bufs=2)
            nc.scalar.copy(gT_bf[:], gT_ps[0:E, :])
            nc.sync.dma_start(gag_in[j][:], gT_bf[:])
            # AllToAll: row e -> rank e; each core receives, from every rank r,
            # the gate row of ITS OWN expert for rank r's tile j.
            nc.gpsimd.collective_compute(
                "AllToAll", OP.bypass, replica_groups=GROUPS_ALL,
                ins=[gag_in[j].opt()], outs=[gag_out[j].opt()])

        # ---------------- Phase B ----------------
        pb = ExitStack()
        pB = pb.enter_context(tc.tile_pool(name="pB", bufs=1))
        pb_mm = pb.enter_context(tc.tile_pool(name="pb_mm", bufs=2,
                                              space="PSUM"))
        pb_st = pb.enter_context(tc.tile_pool(name="pb_st", bufs=2,
                                              space="PSUM"))
        pb_yt = pb.enter_context(tc.tile_pool(name="pb_yt", bufs=2,
                                              space="PSUM"))
        krope = pB.tile([64, T], BF16, name="krope", tag="krope")
        vav = [pB.tile([P, 65], BF16, name=f"vav_{kb}", tag=f"vav{kb}")
               for kb in range(NKB)]
        for kb in range(NKB):
            nc.vector.memset(vav[kb][:, 64:65], 1.0)

        def slab_b(j):
            js = j * 512
            # assemble h1^T slab (bf16) from AG chunk
            h1c = []
            for k in range(CK):
                hc = pB.tile([P, 512], BF16, name=f"h1c_{j}_{k}",
                             tag=f"h1c{k}", bufs=2)
                for r in range(4):
                    nc.sync.dma_start(
                        hc[:, r * P:(r + 1) * P],
                        h1ag_out[j][r * C + k * P: r * C + (k + 1) * P, :])
                h1c.append(hc)
            # K projection + rope
            kv_ps = pb_mm.tile([P, 512], F32, name="kv_ps", tag="mm", bufs=2)
            for k in range(CK):
                nc.tensor.matmul(kv_ps[0:64, :], wkv_sb[k][:, 0:64],
                                 h1c[k][:], start=(k == 0), stop=(k == CK - 1))
            kt = pB.tile([64, 512], BF16, name="kt", tag="kt", bufs=2)
            nc.scalar.copy(kt[:], kv_ps[0:64, :])
            krot_ps = pb_mm.tile([P, 512], F32, name="krot_ps", tag="mm",
                                 bufs=2)
            nc.tensor.matmul(krot_ps[0:64, :], rotp[:], kt[:],
                             start=True, stop=True)
            kc = pB.tile([64, 512], F32, name="kc", tag="kc", bufs=1)
            nc.vector.tensor_mul(kc[:], kt[:], cosq[:, js:js + 512])
            ks = pB.tile([64, 512], F32, name="ks", tag="ks", bufs=1)
            nc.vector.tensor_mul(ks[:], krot_ps[0:64, :],
                                 sinq[:, js:js + 512])
            nc.vector.tensor_add(krope[:, js:js + 512], kc[:], ks[:])
            # V^T directly: vav[4j+i][:, 0:64] = (h1c_block)^T @ wv
            for i in range(4):
                vav_ps = pb_mm.tile([P, 512], F32, name="vav_ps", tag="mm",
                                    bufs=2)
                for k in range(CK):
                    nc.tensor.matmul(vav_ps[:, 0:64],
                                     h1c[k][:, i * P:(i + 1) * P],
                                     wkv_sb[k][:, 64:128],
                                     start=(k == 0), stop=(k == CK - 1))
                nc.scalar.copy(vav[4 * j + i][:, 0:64], vav_ps[:, 0:64])
            # Q projection + rope (4 heads)
            qrope = []
            for h in range(4):
                q_ps = pb_mm.tile([P, 512], F32, name="q_ps", tag="mm",
                                  bufs=2)
                for k in range(CK):
                    nc.tensor.matmul(q_ps[0:64, :],
                                     wq_sb[k][:, h * 64:(h + 1) * 64],
                                     h1c[k][:], start=(k == 0),
                                     stop=(k == CK - 1))
                qt = pB.tile([64, 512], BF16, name="qt", tag="qt", bufs=3)
                nc.scalar.copy(qt[:], q_ps[0:64, :])
                qrot_ps = pb_mm.tile([P, 512], F32, name="qrot_ps", tag="mm",
                                     bufs=2)
                nc.tensor.matmul(qrot_ps[0:64, :], rotp[:], qt[:],
                                 start=True, stop=True)
                qc = pB.tile([64, 512], F32, name="qc", tag="qc", bufs=1)
                nc.vector.tensor_mul(qc[:], qt[:], cosq[:, js:js + 512])
                qs = pB.tile([64, 512], F32, name="qs", tag="qs", bufs=1)
                nc.vector.tensor_mul(qs[:], qrot_ps[0:64, :],
                                     sinq[:, js:js + 512])
                qr = pB.tile([64, 512], BF16, name=f"qr_{h}", tag=f"qr{h}",
                             bufs=2)
                nc.vector.tensor_add(qr[:], qc[:], qs[:])
                qrope.append(qr)
            # attention per head
            ytall = [pB.tile([P, 512], BF16, name=f"ytall_{j}_{i}",
                             tag=f"ytall{i}", bufs=2) for i in range(2)]
            for h in range(4):
                yt = pb_yt.tile([65, 512], F32, name="yt_ps", tag="yt",
                                bufs=2)
                for kb in range(4 * j + 4):
                    qoff = max(0, (kb - 4 * j) * P)
                    w = 512 - qoff
                    st = pb_st.tile([P, 512], F32, name="st_ps", tag="st",
                                    bufs=2)
                    nc.tensor.matmul(st[:, :w],
                                     krope[:, kb * P:(kb + 1) * P],
                                     qrope[h][:, qoff:512],
                                     start=True, stop=True)
                    pexp = pB.tile([P, 512], BF16, name="pexp", tag="pexp",
                                   bufs=4)
                    nc.scalar.activation(pexp[:, :w], st[:, :w], ACT.Exp,
                                         scale=0.125)
                    nc.tensor.matmul(yt[:, qoff:512], vav[kb][:],
                                     pexp[:, :w], start=(kb == 0),
                                     stop=False)
                nc.tensor.matmul(yt[:], zl[:], zr[:], start=False, stop=True)
                lrow = pB.tile([1, 512], F32, name="lrow", tag="lrow", bufs=1)
                nc.vector.tensor_copy(lrow[:], yt[64:65, :])
                rec = pB.tile([1, 512], F32, name="rec", tag="rec", bufs=1)
                recip(pB, rec[:], lrow[:], f"at_{j}_{h}")
                rec_bf = pB.tile([1, 512], BF16, name="rec_bf", tag="rec_bf",
                                 bufs=2)
                nc.vector.tensor_copy(rec_bf[:], rec[:])
                recb = pb_mm.tile([P, 512], F32, name="recb_ps", tag="mm",
                                  bufs=2)
                nc.tensor.matmul(recb[0:64, :], ones1[:], rec_bf[:],
                                 start=True, stop=True)
                ytc = pB.tile([64, 512], BF16, name="ytc", tag="ytc", bufs=2)
                nc.scalar.copy(ytc[:], yt[0:64, :])
                nc.vector.tensor_mul(
                    ytall[h // 2][(h % 2) * 64:(h % 2) * 64 + 64, :],
                    ytc[:], recb[0:64, :])
            # output projection + chunked ReduceScatter
            for tt in range(4):
                for n in range(2):
                    o_ps = pb_mm.tile([P, 512], F32, name="o_ps", tag="mm",
                                      bufs=2)
                    for k in range(2):
                        nc.tensor.matmul(o_ps[:],
                                         ytall[k][:, tt * P:(tt + 1) * P],
                                         wo_sb[k][:, n * 512:(n + 1) * 512],
                                         start=(k == 0), stop=(k == 1))
                    o_bf = pB.tile([P, 512], BF16, name="o_bf", tag="o_bf",
                                   bufs=3)
                    nc.scalar.copy(o_bf[:], o_ps[:])
                    nc.sync.dma_start(
                        rsat_in[j][tt * P:(tt + 1) * P,
                                   n * 512:(n + 1) * 512], o_bf[:])
            nc.gpsimd.collective_compute(
                "ReduceScatter", OP.add, replica_groups=GROUPS_B,
                ins=[rsat_in[j].opt()], outs=[rsat_out[j].opt()])

        slab_b(0)
        slab_b(1)
        slab_b(2)
        phase_c(0)
        slab_b(3)
        pb.close()
        phase_c(1)
        phase_c(2)

        # ---------------- Phase D ----------------
        pd = top.enter_context(tc.tile_pool(name="pd", bufs=1))
        pd_ps = top.enter_context(tc.tile_pool(name="pd_ps", bufs=3,
                                               space="PSUM"))

        def chunk_d(j):
            h2T = []
            for k in range(CK):
                tt = pd.tile([P, 1024], BF16, name=f"h2T_{j}_{k}",
                             tag=f"h2T{k}", bufs=2)
                for r in range(NCORE):
                    nc.sync.dma_start(
                        tt[:, r * P:(r + 1) * P],
                        h2ag_out[j][r * C + k * P: r * C + (k + 1) * P, :])
                h2T.append(tt)
            gbc = pd.tile([P, 1024], BF16, name=f"gbc_{j}", tag="gbc",
                          bufs=1)
            for r in range(NCORE):
                nc.sync.dma_start(
                    gbc[:, r * P:(r + 1) * P],
                    gag_out[j][r, :][None, :].to_broadcast([P, P]))
            # shared expert
            hT = [pd.tile([P, 1024], BF16, name=f"hT_{j}_{m}", tag=f"hT{m}",
                          bufs=2) for m in range(2)]
            for m in range(2):
                for n in range(2):
                    a1 = pd_ps.tile([P, 512], F32, name="a1_ps", tag="d_ps",
                                    bufs=3)
                    for k in range(CK):
                        nc.tensor.matmul(a1[:],
                                         swA_sb[k][:, m * P:(m + 1) * P],
                                         h2T[k][:, n * 512:(n + 1) * 512],
                                         start=(k == 0), stop=(k == CK - 1))
                    stmp = pd.tile([P, 512], BF16, name="stmp", tag="stmp",
                                   bufs=2)
                    nc.scalar.activation(stmp[:], a1[:], ACT.Silu)
                    a3 = pd_ps.tile([P, 512], F32, name="a3_ps", tag="d_ps",
                                    bufs=3)
                    for k in range(CK):
                        nc.tensor.matmul(
                            a3[:], swA_sb[k][:, 256 + m * P:256 + (m + 1) * P],
                            h2T[k][:, n * 512:(n + 1) * 512],
                            start=(k == 0), stop=(k == CK - 1))
                    nc.vector.tensor_mul(hT[m][:, n * 512:(n + 1) * 512],
                                         stmp[:], a3[:])
            # routed expert mid
            hmid = [pd.tile([P, 1024], BF16, name=f"hm_{j}_{m}", tag=f"hm{m}",
                            bufs=1) for m in range(CK)]
            for m in range(CK):
                for n in range(2):
                    w1 = pd_ps.tile([P, 512], F32, name="w1_ps", tag="d_ps",
                                    bufs=3)
                    for k in range(CK):
                        nc.tensor.matmul(w1[:],
                                         rw1_sb[k][:, m * P:(m + 1) * P],
                                         h2T[k][:, n * 512:(n + 1) * 512],
                                         start=(k == 0), stop=(k == CK - 1))
                    gl = pd.tile([P, 512], BF16, name="gl", tag="gl", bufs=2)
                    nc.scalar.activation(gl[:], w1[:], ACT.Gelu)
                    nc.vector.tensor_mul(hmid[m][:, n * 512:(n + 1) * 512],
                                         gl[:], gbc[:, n * 512:(n + 1) * 512])
            # fused output matmul
            for tt in range(NCORE):
                for n in range(2):
                    o2 = pd_ps.tile([P, 512], F32, name="o2_ps", tag="d_ps",
                                    bufs=3)
                    for k in range(2):
                        nc.tensor.matmul(o2[:], hT[k][:, tt * P:(tt + 1) * P],
                                         sw2_sb[k][:, n * 512:(n + 1) * 512],
                                         start=(k == 0), stop=False)
                    for k in range(CK):
                        nc.tensor.matmul(o2[:],
                                         hmid[k][:, tt * P:(tt + 1) * P],
                                         rw2_sb[k][:, n * 512:(n + 1) * 512],
                                         start=False, stop=(k == CK - 1))
                    mo = pd.tile([P, 512], BF16, name="mo", tag="mo", bufs=2)
                    nc.vector.tensor_copy(mo[:], o2[:])
                    nc.sync.dma_start(
                        rsmo_in[j][tt * P:(tt + 1) * P,
                                   n * 512:(n + 1) * 512], mo[:])
            nc.gpsimd.collective_compute(
                "ReduceScatter", OP.add, replica_groups=GROUPS_ALL,
                ins=[rsmo_in[j].opt()], outs=[rsmo_out[j].opt()])
            # final residual for own tile j
            mo_bf = pd.tile([P, C], BF16, name=f"mo_{j}", tag="fmo", bufs=1)
            nc.sync.dma_start(mo_bf[:], rsmo_out[j][:])
            mo32 = pd.tile([P, C], F32, name=f"mo32_{j}", tag="fmo32", bufs=1)
            nc.vector.tensor_copy(mo32[:], mo_bf[:])
            o_sb = pd.tile([P, C], F32, name=f"fo_{j}", tag="fo", bufs=1)
            nc.vector.tensor_add(o_sb[:], x2[j][:], mo32[:])
            nc.sync.dma_start(io["out"][j * P:(j + 1) * P, :], o_sb[:])

        chunk_d(0)
        phase_c(3)
        chunk_d(1)
        chunk_d(2)
        chunk_d(3)


# =============================================================================
# host side
# =============================================================================

def _rope_tables():
    freqs = (1.0 / (THETA ** (np.arange(0, HD, 2, dtype=np.float64) / HD)))
    t = np.arange(T, dtype=np.float64)
    emb = np.outer(t, freqs)                                # [T, 32]
    cos = np.concatenate([np.cos(emb), np.cos(emb)], 1).T   # [64, T]
    sin = np.concatenate([np.sin(emb), np.sin(emb)], 1).T   # [64, T]
    return cos.astype(np.float32), sin.astype(np.float32)


def _shard_inputs(inp):
    bf = ml_dtypes.bfloat16
    f32 = np.float32
    x = np.asarray(inp["x"], f32)                # [B, T, C]
    t_emb = np.asarray(inp["t_emb"], f32)
    ada_cat = np.concatenate([np.asarray(inp["ada1_w"], f32),
                              np.asarray(inp["ada2_w"], f32)], 1)  # [C, 4096]
    adab_cat = np.concatenate([np.asarray(inp["ada1_b"], f32),
                               np.asarray(inp["ada2_b"], f32)])    # [4096]
    wq = np.asarray(inp["wq"], f32)
    wk = np.asarray(inp["wk"], f32)
    wv = np.asarray(inp["wv"], f32)
    wo = np.asarray(inp["wo"], f32)
    sw1 = np.asarray(inp["sw1"], f32)
    sw3 = np.asarray(inp["sw3"], f32)
    sw2 = np.asarray(inp["sw2"], f32)
    rw1 = np.asarray(inp["re_w1"], f32)
    rw2 = np.asarray(inp["re_w2"], f32)
    rtw = np.asarray(inp["router_w"], f32)
    rtb = np.asarray(inp["router_bias"], f32)
    cosq, sinq = _rope_tables()
    ident = np.eye(P, dtype=f32)
    rotp = np.zeros((64, 64), dtype=f32)
    for i in range(32):
        rotp[32 + i, i] = -1.0     # out[p<32] = -q[p+32]
        rotp[i, 32 + i] = 1.0      # out[p>=32] = q[p-32]

    in_maps = []
    for c in range(NCORE):
        b, g = c // 4, c % 4
        xq = np.concatenate(
            [x[b, (4 * j + g) * P:(4 * j + g + 1) * P] for j in range(NT_Q)])
        m = {
            "x_q": np.ascontiguousarray(xq),
            "temb_b": np.ascontiguousarray(t_emb[b].reshape(C, 1)),
            "ada_w_s": np.ascontiguousarray(
                ada_cat[:, g * 1024:(g + 1) * 1024]).astype(bf),
            "ada_b_s": np.ascontiguousarray(
                adab_cat[g * 1024:(g + 1) * 1024].reshape(1, 1024)),
            "wq_s": np.ascontiguousarray(
                wq[:, 256 * g:256 * (g + 1)]).astype(bf),
            "wkv_s": np.ascontiguousarray(np.concatenate(
                [wk[:, 64 * g:64 * (g + 1)],
                 wv[:, 64 * g:64 * (g + 1)]], 1)).astype(bf),
            "wo_s": np.ascontiguousarray(
                wo[256 * g:256 * (g + 1), :]).astype(bf),
            "cosq": cosq,
            "sinq": sinq,
            "identf": ident,
            "rotp": rotp.astype(bf),
            "swA_s": np.ascontiguousarray(np.concatenate(
                [sw1[:, 256 * c:256 * (c + 1)],
                 sw3[:, 256 * c:256 * (c + 1)]], 1)).astype(bf),
            "sw2_s": np.ascontiguousarray(
                sw2[256 * c:256 * (c + 1), :]).astype(bf),
            "rw1_e": np.ascontiguousarray(rw1[c]).astype(bf),
            "rw2_e": np.ascontiguousarray(rw2[c]).astype(bf),
            "router_w": rtw,
            "router_bias": rtb.reshape(1, E),
        }
        in_maps.append(m)
    return in_maps


_NC_CACHE = []


def _install_ntff_hook():
    """Provide antenv.axon_hooks (absent in this image) so trace=True works."""
    import sys
    import types
    try:
        import antenv
        if "antenv.axon_hooks" not in sys.modules:
            mod = types.ModuleType("antenv.axon_hooks")
            mod._hook = None

            def set_axon_ntff_profile_hook(h):
                mod._hook = h

            def get_axon_ntff_profile_hook():
                return mod._hook

            mod.set_axon_ntff_profile_hook = set_axon_ntff_profile_hook
            mod.get_axon_ntff_profile_hook = get_axon_ntff_profile_hook
            sys.modules["antenv.axon_hooks"] = mod
            antenv.axon_hooks = mod
        mod = sys.modules["antenv.axon_hooks"]
        if mod.get_axon_ntff_profile_hook() is None:
            from trn_agent_boot.trn_boot import _ntff_profile_via_ctypes
            hook = _ntff_profile_via_ctypes("/opt/axon/libaxon_pjrt.so")
            if hook is not None:
                mod.set_axon_ntff_profile_hook(hook)
        import concourse.bass_utils as bu
        bu.upload_artifacts = lambda d: d
        return True
    except Exception:
        return False


def kernel(**inputs):
    global LAST_EXEC_NS
    if not _NC_CACHE:
        _NC_CACHE.append(build_program())
    nc = _NC_CACHE[0]
    in_maps = _shard_inputs(inputs)
    trace = bool(int(os.environ.get("KB_TRACE", "0")))
    if trace:
        trace = _install_ntff_hook()
    res = None
    if trace:
        try:
            res = run_bass_kernel_spmd(nc, in_maps,
                                       core_ids=list(range(NCORE)),
                                       trace=True,
                                       tmpdir=os.environ.get("KB_TRACE_DIR"))
        except Exception as e:
            print(f"traced run failed ({e!r}); falling back to untraced")
            res = None
    if res is None:
        res = run_bass_kernel_spmd(nc, in_maps, core_ids=list(range(NCORE)))
    LAST_EXEC_NS = res.exec_time_ns
    out = np.empty((B, T, C), np.float32)
    for c in range(NCORE):
        b, g = c // 4, c % 4
        oc = res.results[c]["out"].astype(np.float32)
        for j in range(NT_Q):
            out[b, (4 * j + g) * P:(4 * j + g + 1) * P] = \
                oc[j * P:(j + 1) * P]
    return out
